# revision 29
# baseline (speedup 1.0000x reference)
# Trainium2 Bass kernel for FC_STGNN pedestrian edge-MLP (gnn_message_passing).
#
# Reference computation (BS=4, N=512, EMB=64):
#   h    = leaky(nf @ fc1_w + fc1_b)            [4,512,128]
#   emb  = leaky(h @ fc2_w + fc2_b)             [4,512,64]
#   edge_in[b,i,j] = [emb_i - emb_j, emb_i*emb_j]
#   eh   = relu(edge_in @ e1_w + e1_b)          [4,512,512,64]
#   logits = eh @ e2_w + e2_b                   [4,512,512,2]
#   edge_prob = softmax(logits)                 -> adjacency = p1, prediction = off-diag (p0,p1)
#
# Kernel restructuring:
#   * softmax over 2 classes == sigmoid of the logit difference:
#       d = eh @ (e2_w[:,1]-e2_w[:,0]) + (e2_b[1]-e2_b[0]);  p1 = sigmoid(d), p0 = sigmoid(-d)
#   * pre-activation factorization (W1 = e1_w[:64], W2 = e1_w[64:]):
#       pre[i,j,:] = emb_j @ (diag(emb_i) @ W2 - W1) + (emb_i @ W1 + e1_b)
#     Per output row i a [64,64] weight block is built on Pool/DVE, then a bf16
#     matmul against the shared embT computes pre for all 512 j. Four rows run
#     per PE "burst" via 2x2 tile_position quadrant packing (concurrent 64x64
#     sub-arrays). The per-i constant rides as the relu's per-partition bias
#     (C2 stacked-bias trick: C2[0:64,c]=C(c), C2[64:128,c]=C(c+1)).
#   * the logit reduction uses eh as the (bf16, fast-weight-load) stationary
#     operand so per-row logits land as columns of a shared PSUM bank; sigmoid
#     is batched over 128 rows and PE-transposed back to row-major chunks.
#
# Sharding: 8 cores; core c -> batch b=c//2, i-rows [i0, i0+256) with i0=256*(c%2).
# The per-core program is identical: the j axis is ROTATED by -i0 on the host
# (nfT input is np.roll'ed), so device columns j' correspond to real j=(j'+i0)%512
# and the core's own i-rows are always device columns 0..255. Host un-rotates.

import numpy as np

BS, N, EMB = 4, 512, 64
IN_DIM, HID = 256, 128
NCORES = 8
HALF = N // 2  # rows per core

_rows, _cols = np.nonzero(~np.eye(N, dtype=bool))

_COMPILED = {}


def _build_program(loop_n=1):
    import contextlib

    import concourse.tile as tile
    from concourse import bacc, mybir

    dt = mybir.dt
    f32 = dt.float32
    f32r = dt.float32r
    bf16 = dt.bfloat16
    AF = mybir.ActivationFunctionType
    OP = mybir.AluOpType

    nc = bacc.Bacc("TRN2", target_bir_lowering=False, debug=False)

    # ---- DRAM I/O (per-core shapes) ----
    nfT0 = nc.dram_tensor("nfT0", [128, N], f32, kind="ExternalInput").ap()
    nfT1 = nc.dram_tensor("nfT1", [128, N], f32, kind="ExternalInput").ap()
    fc1w0 = nc.dram_tensor("fc1w0", [128, HID], f32, kind="ExternalInput").ap()
    fc1w1 = nc.dram_tensor("fc1w1", [128, HID], f32, kind="ExternalInput").ap()
    fc2w = nc.dram_tensor("fc2w", [HID, EMB], f32, kind="ExternalInput").ap()
    # w1d/w2d: e1_w halves, host-duplicated to both partition halves [128, 64]
    w1d = nc.dram_tensor("w1d", [128, EMB], f32, kind="ExternalInput").ap()
    w2d = nc.dram_tensor("w2d", [128, EMB], f32, kind="ExternalInput").ap()
    # brow[0, 0:128]=fc1_b, [128:192]=fc2_b, [192:256]=e1_b
    brow = nc.dram_tensor("brow", [1, 256], f32, kind="ExternalInput").ap()
    # aux cols: 2=b_diff, 3=-b_diff (replicated down partitions)
    aux = nc.dram_tensor("aux", [128, 4], f32, kind="ExternalInput").ap()
    # aux2: [wd;0] and [0;wd] in bf16 (stationary side of the logit reduction)
    aux2 = nc.dram_tensor("aux2", [128, 2], bf16, kind="ExternalInput").ap()
    eye = nc.dram_tensor("eye", [128, 128], f32, kind="ExternalInput").ap()

    a_out = nc.dram_tensor("a_out", [HALF, N], f32, kind="ExternalOutput").ap()
    p0_out = nc.dram_tensor("p0_out", [HALF, N], f32, kind="ExternalOutput").ap()
    emb_out = nc.dram_tensor("emb_out", [HALF, EMB], f32, kind="ExternalOutput").ap()

    with tile.TileContext(nc) as tc:
        loop = tc.For_i(0, loop_n, 1) if loop_n > 1 else contextlib.nullcontext()
        with (
            loop,
            tc.tile_pool(name="const", bufs=1) as cpool,
            tc.tile_pool(name="wquad", bufs=6) as wpool,
            tc.tile_pool(name="eh", bufs=6) as ehpool,
            tc.tile_pool(name="sig", bufs=2) as sigpool,
            tc.tile_pool(name="chunk", bufs=2) as chpool,
            tc.tile_pool(name="psA", bufs=3, space="PSUM") as psA,
            tc.tile_pool(name="psL", bufs=1, space="PSUM") as psL,
            tc.tile_pool(name="psM", bufs=1, space="PSUM") as psM,
        ):
            # ---- load constants ----
            t_nfT0 = cpool.tile([128, N], f32, tag="nfT0")
            t_nfT1 = cpool.tile([128, N], f32, tag="nfT1")
            t_fc1w0 = cpool.tile([128, HID], f32, tag="fc1w0")
            t_fc1w1 = cpool.tile([128, HID], f32, tag="fc1w1")
            t_fc2w = cpool.tile([HID, EMB], f32, tag="fc2w")
            t_w1d = cpool.tile([128, EMB], f32, tag="w1d")
            t_w2d = cpool.tile([128, EMB], f32, tag="w2d")
            t_brow = cpool.tile([1, 256], f32, tag="brow")
            t_aux = cpool.tile([128, 4], f32, tag="aux")
            t_aux2 = cpool.tile([128, 2], bf16, tag="aux2")
            t_eye = cpool.tile([128, 128], f32, tag="eye")
            nc.sync.dma_start(t_nfT0[:], nfT0)
            nc.sync.dma_start(t_fc1w0[:], fc1w0)
            nc.sync.dma_start(t_nfT1[:], nfT1)
            nc.sync.dma_start(t_fc1w1[:], fc1w1)
            nc.sync.dma_start(t_fc2w[:], fc2w)
            nc.sync.dma_start(t_w1d[:], w1d)
            nc.sync.dma_start(t_w2d[:], w2d)
            nc.sync.dma_start(t_brow[:], brow)
            nc.sync.dma_start(t_aux[:], aux)
            nc.sync.dma_start(t_aux2[:], aux2)
            nc.sync.dma_start(t_eye[:], eye)

            # DVE/Pool conversion copies: every DMA-fed fp32(r) matmul operand
            # gets a single-engine producer (those matmuls support only ONE
            # semaphore wait) and prep matmuls run at f32r speed.
            t_nfT0r = cpool.tile([128, N], f32r, tag="nfT0r")
            t_nfT1r = cpool.tile([128, N], f32r, tag="nfT1r")
            t_fc1w0r = cpool.tile([128, HID], f32r, tag="fc1w0r")
            t_fc1w1r = cpool.tile([128, HID], f32r, tag="fc1w1r")
            t_fc2wr = cpool.tile([HID, EMB], f32r, tag="fc2wr")
            t_w1r = cpool.tile([EMB, EMB], f32r, tag="w1r")
            t_browr = cpool.tile([1, 256], f32r, tag="browr")
            nc.vector.tensor_copy(t_fc1w0r[:], t_fc1w0[:])
            nc.vector.tensor_copy(t_nfT0r[:], t_nfT0[:])
            nc.gpsimd.tensor_copy(t_fc1w1r[:], t_fc1w1[:])
            nc.gpsimd.tensor_copy(t_nfT1r[:], t_nfT1[:])
            nc.vector.tensor_copy(t_fc2wr[:], t_fc2w[:])
            nc.gpsimd.tensor_copy(t_w1r[:], t_w1d[0:EMB, :])
            nc.gpsimd.tensor_copy(t_browr[:], t_brow[:])
            nc.gpsimd.tensor_copy(t_eye[:], t_eye[:])
            nc.gpsimd.tensor_copy(t_aux2[:], t_aux2[:])

            t_onesf = cpool.tile([1, N], f32, tag="onesf")
            nc.vector.memset(t_onesf[:], 1.0)
            t_ones = cpool.tile([1, N], f32r, tag="ones")
            nc.vector.tensor_copy(t_ones[:], t_onesf[:])

            def leaky(dst, src, scratch_pool):
                # max(x, 0.01x); only one PSUM operand allowed per instruction,
                # so stage 0.01x through SBUF first.
                t_s = scratch_pool.tile(
                    [src.partition_size(), src.free_size()], f32, tag="leak"
                )
                nc.vector.tensor_scalar_mul(t_s[:], src, 0.01)
                nc.vector.scalar_tensor_tensor(
                    dst, src, 0.0, t_s[:], op0=OP.add, op1=OP.max
                )

            # ---- fc head: hT = leaky(fc1_w.T @ nfT + fc1_b) ----
            ps_h = psM.tile([128, N], f32, tag="prep")
            nc.tensor.matmul(ps_h[:], t_fc1w0r[:], t_nfT0r[:], start=True, stop=False)
            nc.tensor.matmul(ps_h[:], t_fc1w1r[:], t_nfT1r[:], start=False, stop=False)
            nc.tensor.matmul(
                ps_h[:], t_browr[0:1, 0:128], t_ones[:], start=False, stop=True
            )
            t_hT = cpool.tile([128, N], f32r, tag="hT")
            leaky(t_hT[:], ps_h[:], ehpool)

            # ---- embT = leaky(fc2_w.T @ hT + fc2_b) ----
            # f32r copy duplicated to both partition halves (build scalars) and
            # a bf16 duplicated copy (matmul rhs for the 2x2 quadrant bursts).
            ps_e = psM.tile([EMB, N], f32, tag="prep")
            nc.tensor.matmul(ps_e[:], t_fc2wr[:], t_hT[:], start=True, stop=False)
            nc.tensor.matmul(
                ps_e[:], t_browr[0:1, 128:192], t_ones[:], start=False, stop=True
            )
            t_embTd = cpool.tile([128, N], f32r, tag="embTd")
            leaky(t_embTd[0:EMB, :], ps_e[:], ehpool)
            t_embTb = cpool.tile([128, N], bf16, tag="embTb")
            nc.vector.tensor_copy(t_embTb[0:EMB, :], t_embTd[0:EMB, :])
            nc.sync.dma_start(t_embTd[EMB:128, :], t_embTd[0:EMB, :])
            nc.sync.dma_start(t_embTb[EMB:128, :], t_embTb[0:EMB, :])

            # ---- C2: relu-bias source. C2[0:64,c]=C(col c), C2[64:128,c]=C(col c+1)
            # where C(col) = W1.T @ embT[:,col] + e1_b ----
            ps_c = psM.tile([EMB, N], f32, tag="prep")
            nc.tensor.matmul(
                ps_c[:], t_w1r[:], t_embTd[0:EMB, :], start=True, stop=False
            )
            nc.tensor.matmul(
                ps_c[:], t_browr[0:1, 192:256], t_ones[:], start=False, stop=True
            )
            t_C2 = cpool.tile([128, N], f32, tag="C2")
            nc.scalar.copy(t_C2[0:EMB, :], ps_c[:])
            nc.sync.dma_start(t_C2[EMB:128, 0 : N - 1], t_C2[0:EMB, 1:N])

            # ---- emb output rows (this core's 256 rows = device cols 0..255) ----
            for k in range(2):
                ps_m = psM.tile([128, EMB], f32, tag="prep")
                nc.tensor.matmul(
                    ps_m[:],
                    t_hT[:, 128 * k : 128 * (k + 1)],
                    t_fc2wr[:],
                    start=True,
                    stop=False,
                )
                nc.tensor.matmul(
                    ps_m[:],
                    t_ones[0:1, 0:128],
                    t_browr[0:1, 128:192],
                    start=False,
                    stop=True,
                )
                t_m = ehpool.tile([128, EMB], f32, tag="embrow")
                leaky(t_m[:], ps_m[:], ehpool)
                nc.sync.dma_start(emb_out[128 * k : 128 * (k + 1), :], t_m[:])

            # ---- main loop: 64 bursts of 4 i-rows, in 2 groups of 32 ----
            NBURST = HALF // 4
            GROUP = 32  # bursts per sigmoid batch (128 rows)
            for g in range(NBURST // GROUP):
                # logitsT bank: column 128*jb + iloc + s = logit(i_loc) for
                # j rows 128*jb..128*jb+127
                ps_log = psL.tile([128, N], f32, tag="logT")
                for bt in range(GROUP):
                    t = g * GROUP + bt
                    i4 = 4 * t  # device column of first row of the burst
                    iloc = 4 * bt

                    # build the 4 weight blocks: wp[:, 64q:64q+64] for
                    # i = i4 + q.  q>0 blocks run on Pool, q=0 on DVE.
                    t_wq = wpool.tile([EMB, 256], bf16, tag="wquad")
                    t_tmp = wpool.tile([EMB, 256], f32, tag="wtmp")
                    for q in range(4):
                        i = i4 + q
                        cs = slice(64 * q, 64 * q + 64)
                        if q > 0:
                            bc = t_embTd[0:EMB, i : i + 1].broadcast_to([EMB, EMB])
                            nc.gpsimd.tensor_tensor(
                                t_tmp[:, cs], t_w2d[0:EMB, :], bc, op=OP.mult
                            )
                            nc.gpsimd.tensor_tensor(
                                t_wq[:, cs], t_tmp[:, cs], t_w1d[0:EMB, :],
                                op=OP.subtract,
                            )
                        else:
                            nc.vector.scalar_tensor_tensor(
                                t_wq[:, cs],
                                t_w2d[0:EMB, :],
                                t_embTd[0:EMB, i : i + 1],
                                t_w1d[0:EMB, :],
                                op0=OP.mult,
                                op1=OP.subtract,
                            )

                    # pair matmuls: bank A (cols 0:N) = rows i4, i4+1; bank B
                    # (cols N:2N) = rows i4+2, i4+3
                    ps2 = psA.tile([128, 2 * N], f32, tag="pre")
                    nc.tensor.matmul(ps2[:, 0:N], t_wq[:, 0:128], t_embTb[0:EMB, :])
                    nc.tensor.matmul(
                        ps2[:, N : 2 * N], t_wq[:, 128:256], t_embTb[0:EMB, :]
                    )

                    # relu + stacked bias, one per bank; bf16 output feeds the
                    # fast-weight-load logit reduction
                    for bk in range(2):
                        t_eh = ehpool.tile([128, N], bf16, tag="eh")
                        col = i4 + 2 * bk
                        if bk == 0 or t % 8 == 0:
                            nc.scalar.activation(
                                t_eh[:],
                                ps2[:, bk * N : (bk + 1) * N],
                                AF.Relu,
                                bias=t_C2[:, col : col + 1],
                                scale=1.0,
                            )
                        else:
                            nc.vector.tensor_scalar(
                                t_eh[:],
                                ps2[:, bk * N : (bk + 1) * N],
                                t_C2[:, col : col + 1],
                                0.0,
                                op0=OP.add,
                                op1=OP.max,
                            )
                        base = iloc + 2 * bk
                        for jb in range(4):
                            nc.tensor.matmul(
                                ps_log[:, 128 * jb + base : 128 * jb + base + 2],
                                t_eh[:, 128 * jb : 128 * (jb + 1)],
                                t_aux2[:],
                            )

                # batched sigmoid over the whole group (still j-major)
                t_p1T = sigpool.tile([128, N], f32, tag="p1T")
                nc.scalar.activation(
                    t_p1T[:], ps_log[:], AF.Sigmoid, bias=t_aux[:, 2:3], scale=1.0
                )

                # transpose back to row-major [i_loc, j]; p0 = 1 - p1 exactly
                # (the reference softmax obeys the same identity)
                t_chunk1 = chpool.tile([128, N], f32, tag="chunk1")
                for jb in range(4):
                    ps_t = psM.tile([128, 128], f32, tag="prep")
                    nc.tensor.transpose(
                        ps_t[:], t_p1T[:, 128 * jb : 128 * (jb + 1)], t_eye[:]
                    )
                    if jb % 2 == 0:
                        nc.vector.tensor_copy(
                            t_chunk1[:, 128 * jb : 128 * (jb + 1)], ps_t[:]
                        )
                    else:
                        nc.scalar.copy(
                            t_chunk1[:, 128 * jb : 128 * (jb + 1)], ps_t[:]
                        )
                t_chunk0 = chpool.tile([128, N], f32, tag="chunk0")
                nc.vector.tensor_scalar(
                    t_chunk0[:], t_chunk1[:], -1.0, 1.0, op0=OP.mult, op1=OP.add
                )
                nc.sync.dma_start(a_out[128 * g : 128 * (g + 1), :], t_chunk1[:])
                nc.sync.dma_start(p0_out[128 * g : 128 * (g + 1), :], t_chunk0[:])

    nc.compile()
    return nc


def _get_program(loop_n=1):
    key = ("nc", loop_n)
    if key not in _COMPILED:
        _COMPILED[key] = _build_program(loop_n)
    return _COMPILED[key]


def _make_in_maps(inputs):
    import ml_dtypes

    nf = np.asarray(inputs["node_features"], np.float32)
    fc1_w = np.asarray(inputs["fc1_w"], np.float32)
    fc1_b = np.asarray(inputs["fc1_b"], np.float32)
    fc2_w = np.asarray(inputs["fc2_w"], np.float32)
    fc2_b = np.asarray(inputs["fc2_b"], np.float32)
    e1_w = np.asarray(inputs["e1_w"], np.float32)
    e1_b = np.asarray(inputs["e1_b"], np.float32)
    e2_w = np.asarray(inputs["e2_w"], np.float32)
    e2_b = np.asarray(inputs["e2_b"], np.float32)

    wd = e2_w[:, 1] - e2_w[:, 0]  # [64]
    b_diff = float(e2_b[1] - e2_b[0])

    brow = np.zeros((1, 256), np.float32)
    brow[0, 0:128] = fc1_b
    brow[0, 128:192] = fc2_b
    brow[0, 192:256] = e1_b

    aux = np.zeros((128, 4), np.float32)
    aux[:, 2] = b_diff
    aux[:, 3] = -b_diff

    aux2 = np.zeros((128, 2), np.float32)
    aux2[0:64, 0] = wd
    aux2[64:128, 1] = wd
    aux2 = aux2.astype(ml_dtypes.bfloat16)

    common = {
        "fc1w0": np.ascontiguousarray(fc1_w[0:128]),
        "fc1w1": np.ascontiguousarray(fc1_w[128:256]),
        "fc2w": np.ascontiguousarray(fc2_w),
        "w1d": np.ascontiguousarray(np.concatenate([e1_w[0:64], e1_w[0:64]], 0)),
        "w2d": np.ascontiguousarray(np.concatenate([e1_w[64:128], e1_w[64:128]], 0)),
        "brow": brow,
        "aux": aux,
        "aux2": aux2,
        "eye": np.eye(128, dtype=np.float32),
    }

    in_maps = []
    for c in range(NCORES):
        b, i0 = c // 2, HALF * (c % 2)
        nfT = np.ascontiguousarray(nf[b].T)  # [256, 512]
        if i0:
            nfT = np.ascontiguousarray(np.roll(nfT, -i0, axis=1))
        m = dict(common)
        m["nfT0"] = np.ascontiguousarray(nfT[0:128])
        m["nfT1"] = np.ascontiguousarray(nfT[128:256])
        in_maps.append(m)
    return in_maps


def _assemble(results):
    adjacency = np.empty((BS, N, N), np.float32)
    p0 = np.empty((BS, N, N), np.float32)
    emb = np.empty((BS, N, EMB), np.float32)
    for c in range(NCORES):
        b, i0 = c // 2, HALF * (c % 2)
        ac = np.asarray(results[c]["a_out"])
        pc = np.asarray(results[c]["p0_out"])
        if i0:
            ac = np.roll(ac, i0, axis=1)
            pc = np.roll(pc, i0, axis=1)
        adjacency[b, i0 : i0 + HALF] = ac
        p0[b, i0 : i0 + HALF] = pc
        emb[b, i0 : i0 + HALF] = np.asarray(results[c]["emb_out"])
    pred = np.stack([p0[:, _rows, _cols], adjacency[:, _rows, _cols]], axis=-1)
    prediction = np.ascontiguousarray(pred.reshape(BS, -1))
    return adjacency, prediction, emb


def kernel(**inputs):
    from concourse import bass_utils

    nc = _get_program()
    in_maps = _make_in_maps(inputs)
    res = bass_utils.run_bass_kernel_spmd(nc, in_maps, core_ids=list(range(NCORES)))
    return _assemble(res.results)
